# revision 20
# baseline (speedup 1.0000x reference)
"""Multi-head attention (B=2, S=2048, D=1024, 16 heads) on 8 Trainium2 NeuronCores.

Sharding: batch x head-group (data + tensor parallel). Core c handles batch
b = c // 4 and head group g = c % 4 (4 heads, feature columns g*256:(g+1)*256
of Wq/Wk/Wv column-split; rows g*256:(g+1)*256 of Wo row-split). Each core
computes a full [2048, 1024] partial of the output projection for its batch;
the host sums the 4 partials per batch (the Wo row-split reduction). The host
ships activations in feature-major ([D, S]) layout - the layout/sharding prep
done in kernel() alongside the per-core slicing - so the device contracts
over the partition dim directly.

Per-core dataflow (matmuls fp32r at full PE rate, fp32 PSUM accumulation):
  K^T, V (natural, ones-augmented), Q^T projections (K/V first so attention
  on early q-blocks overlaps the Q-projection tail)  ->  scores^T = K @ Q^T
  per head (row-packed pairs: two dh=64 heads share the 128-row PE array)
  ->  exp on ACT straight out of PSUM (1/sqrt(dh) fused into the activation
  scale)  ->  Y^T = [V | 1]^T @ expS^T (PSUM-accumulated; row 64 = softmax
  denominator)  ->  normalize by broadcast reciprocal  ->  out = Y @ Wo with
  Y^T stationary (natural output layout)  ->  DMA out.
"""

import os
import sys

for _p in ("/opt/trn_rl_repo", os.path.expanduser("~/.axon_site/_ro/trn_rl_repo")):
    if os.path.isdir(_p) and _p not in sys.path:
        sys.path.append(_p)

import numpy as np

import concourse.bass as bass
import concourse.tile as tile
from concourse import bacc, mybir
from concourse.bass_utils import run_bass_kernel_spmd
from concourse.masks import make_identity

F32 = mybir.dt.float32
FR = mybir.dt.float32r
BF = mybir.dt.bfloat16

S = 2048          # sequence length
D = 1024          # model dim
C = 256           # per-core feature columns (4 heads x 64)
H = 4             # local heads
DH = 64           # head dim
P = 128
NKB = D // P      # 8 contraction blocks for projections
NKT = S // P      # 16 key tiles
NQB = 4           # q blocks of 512
QB = S // NQB     # 512
NSB = 4           # s-blocks for the projection phase
SB = S // NSB     # 512
N_CORES = 8


def _build_program():
    nc = bacc.Bacc("TRN2", target_bir_lowering=False, debug=False,
                   num_devices=N_CORES)
    # activations arrive feature-major: [D, S]
    xq = nc.dram_tensor("xq", [D, S], F32, kind="ExternalInput").ap()
    xk = nc.dram_tensor("xk", [D, S], F32, kind="ExternalInput").ap()
    xv = nc.dram_tensor("xv", [D, S], F32, kind="ExternalInput").ap()
    wq = nc.dram_tensor("wq", [D, C], F32, kind="ExternalInput").ap()
    wk = nc.dram_tensor("wk", [D, C], F32, kind="ExternalInput").ap()
    wv = nc.dram_tensor("wv", [D, C], F32, kind="ExternalInput").ap()
    wo = nc.dram_tensor("wo", [C, D], F32, kind="ExternalInput").ap()
    out = nc.dram_tensor("out", [S, D], F32, kind="ExternalOutput").ap()

    with tile.TileContext(nc) as tc:
        _mha_tile(tc, xq, xk, xv, wq, wk, wv, wo, out)
    nc.compile()
    return nc


def _mha_tile(tc, xq, xk, xv, wq, wk, wv, wo, out):
    nc = tc.nc
    VAR = os.environ.get("KVARIANT", "full")
    _prec = os.environ.get("KPREC", "fp32r")
    AT = FR if _prec == "fp32r" else BF          # AV path: v_sb + expS
    ST = BF if _prec == "bf16" else FR           # scores path: qT/kT
    PT = BF if _prec in ("mixedp", "bf16") else FR   # V-proj + out-proj path
    reps = int(VAR[1:]) if VAR.startswith("x") else 1
    import contextlib
    _fr_all = _prec == "fp32r"
    ctx = contextlib.ExitStack()
    with ctx:
        consts = ctx.enter_context(tc.tile_pool(name="consts", bufs=1))
        xtp = ctx.enter_context(
            tc.tile_pool(name="xtp", bufs=3 if _fr_all else 4))
        acts = ctx.enter_context(tc.tile_pool(name="acts", bufs=1))
        expp = ctx.enter_context(
            tc.tile_pool(name="expp", bufs=3 if _fr_all else 6))
        outp = ctx.enter_context(tc.tile_pool(name="outp", bufs=3))
        small = ctx.enter_context(tc.tile_pool(name="small", bufs=2))
        # PSUM: tag "big" = 3 banks x 2 slots, tag "y" = 1 bank x 2 slots -> 8.
        psb = ctx.enter_context(tc.tile_pool(name="psb", bufs=2, space="PSUM"))
        psy = ctx.enter_context(tc.tile_pool(name="psy", bufs=2, space="PSUM"))

        def big_psum():
            return psb.tile([128, 3, 512], F32, tag="big", name="bigp")

        def y_psum():
            return psy.tile([128, 512], F32, tag="y", name="yp")

        # ---- weights to SBUF
        wq_sb = consts.tile([P, NKB, C], FR, tag="wq")
        nc.sync.dma_start(wq_sb[:],
                          wq.bitcast(FR).rearrange("(kb p) c -> p kb c", p=P))
        wk_sb = consts.tile([P, NKB, C], FR, tag="wk")
        nc.sync.dma_start(wk_sb[:],
                          wk.bitcast(FR).rearrange("(kb p) c -> p kb c", p=P))
        wv_sb = consts.tile([P, NKB, C], PT, tag="wv")
        if PT == FR:
            nc.sync.dma_start(
                wv_sb[:], wv.bitcast(FR).rearrange("(kb p) c -> p kb c", p=P))
        else:
            nc.gpsimd.dma_start(
                wv_sb[:], wv.rearrange("(kb p) c -> p kb c", p=P))
        wo_sb = consts.tile([P, 2, D], PT, tag="wo")
        if PT == FR:
            nc.sync.dma_start(
                wo_sb[:], wo.bitcast(FR).rearrange("(ct p) n -> p ct n", p=P))
        else:
            nc.gpsimd.dma_start(
                wo_sb[:], wo.rearrange("(ct p) n -> p ct n", p=P))

        # ---- persistent activations
        qT = acts.tile([P, 2, S], ST, tag="qT")     # Q^T: [c%128, ct, s]
        kT = acts.tile([P, 2, S], ST, tag="kT")     # K^T
        v_sb = acts.tile([P, NKT, H, DH + 1], AT, tag="v")  # V natural + ones
        yT = acts.tile([P, 2, S], PT, tag="yT")     # Y^T: [c%128, ct, q]
        ones = consts.tile([P, 1], F32, tag="ones")
        nc.vector.memset(ones[:], 1.0)
        nc.vector.tensor_copy(
            v_sb[:, :, :, DH:DH + 1],
            ones[:, :, None].to_broadcast((P, NKT * H, 1)).rearrange(
                "p (t h) o -> p t h o", h=H),
        )

        loop_n = int(VAR[1:]) if VAR.startswith("L") else 0
        loop_cm = tc.For_i(0, loop_n, 1) if loop_n else None
        if loop_cm is not None:
            ctx.enter_context(loop_cm)
        for _rep in range(reps):
            # ---- phase 1: projections (K, V, then Q; x^T streamed from HBM)
            for t, x_d in ((1, xk), (2, xv), (0, xq)):
                for sb in range(NSB):
                    ss = slice(sb * SB, (sb + 1) * SB)
                    xdt = PT if t == 2 else FR
                    xT = xtp.tile([P, NKB, SB], xdt, tag="xT")
                    if xdt == FR:
                        eng = nc.sync if sb % 2 == 0 else nc.scalar
                        eng.dma_start(
                            xT[:],
                            x_d.bitcast(FR).rearrange(
                                "(kb p) s -> p kb s", p=P)[:, :, ss],
                        )
                    else:
                        nc.gpsimd.dma_start(
                            xT[:],
                            x_d.rearrange(
                                "(kb p) s -> p kb s", p=P)[:, :, ss],
                        )
                    if t < 2:  # Q^T / K^T : feature-major
                        dst = qT if t == 0 else kT
                        w_sb = wq_sb if t == 0 else wk_sb
                        for ct in range(2):
                            pq = big_psum()
                            for kb in range(NKB):
                                nc.tensor.matmul(
                                    pq[:, 0, :],
                                    w_sb[:, kb, ct * P:(ct + 1) * P],
                                    xT[:, kb, :],
                                    start=(kb == 0),
                                    stop=(kb == NKB - 1),
                                )
                            nc.any.tensor_copy(dst[:, ct, ss], pq[:, 0, :])
                    else:  # V natural, scattered into per-head 65-col slots
                        for st in range(4):
                            pv = big_psum()
                            for kb in range(NKB):
                                nc.tensor.matmul(
                                    pv[:, 0, 0:C],
                                    xT[:, kb, st * P:(st + 1) * P],
                                    wv_sb[:, kb, :],
                                    start=(kb == 0),
                                    stop=(kb == NKB - 1),
                                )
                            nc.any.tensor_copy(
                                v_sb[:, sb * 4 + st, :, 0:DH],
                                pv[:, 0, 0:C].rearrange(
                                    "p (h d) -> p h d", h=H),
                            )

            # ---- phase 2: attention + output projection
            KT_GROUPS = [(0, 3), (3, 6), (6, 9), (9, 12), (12, 15), (15, 16)]
            for qb in range(NQB):
                qs = slice(qb * QB, (qb + 1) * QB)
                for pair in range(2):  # heads (2*pair, 2*pair+1)
                    ya = y_psum()
                    yb = y_psum()
                    for g0, g1 in KT_GROUPS:
                        gn = g1 - g0
                        sa = big_psum()
                        sb_ = big_psum()
                        for j, kt in enumerate(range(g0, g1)):
                            ks = slice(kt * P, (kt + 1) * P)
                            nc.tensor.matmul(
                                sa[:, j, :], kT[0:DH, pair, ks],
                                qT[0:DH, pair, qs],
                                start=True, stop=True, tile_position=(0, 0),
                            )
                            nc.tensor.matmul(
                                sb_[:, j, :], kT[DH:P, pair, ks],
                                qT[DH:P, pair, qs],
                                start=True, stop=True, tile_position=(64, 0),
                            )
                        ea = expp.tile([P, 3, 512], AT, tag="exp")
                        eb = expp.tile([P, 3, 512], AT, tag="exp")
                        if VAR == "noexp":
                            nc.vector.tensor_copy(ea[:, 0:gn, :],
                                                  sa[:, 0:gn, :])
                            nc.vector.tensor_copy(eb[:, 0:gn, :],
                                                  sb_[:, 0:gn, :])
                        else:
                            nc.scalar.activation(
                                ea[:, 0:gn, :], sa[:, 0:gn, :],
                                mybir.ActivationFunctionType.Exp, scale=0.125)
                            nc.scalar.activation(
                                eb[:, 0:gn, :], sb_[:, 0:gn, :],
                                mybir.ActivationFunctionType.Exp, scale=0.125)
                        for j, kt in enumerate(range(g0, g1)):
                            nc.tensor.matmul(
                                ya[0:DH + 1, :],
                                v_sb[:, kt, 2 * pair, :],
                                ea[:, j, :],
                                start=(kt == 0), stop=(kt == NKT - 1),
                            )
                            nc.tensor.matmul(
                                yb[0:DH + 1, :],
                                v_sb[:, kt, 2 * pair + 1, :],
                                eb[:, j, :],
                                start=(kt == 0), stop=(kt == NKT - 1),
                            )
                    # normalize: rows 0..63 / broadcast(row 64)
                    for a, yp in ((0, ya), (1, yb)):
                        rc = small.tile([1, 512], F32, tag="rc")
                        nc.vector.reciprocal(rc[:], yp[DH:DH + 1, :])
                        rcb = small.tile([DH, 512], F32, tag="rcb")
                        nc.gpsimd.partition_broadcast(rcb[:], rc[:])
                        nc.vector.tensor_mul(
                            yT[a * DH:(a + 1) * DH, pair, qs],
                            yp[0:DH, :], rcb[:])

                # output projection for this q block (both pairs now done)
                for qt in range(qb * 4, (qb + 1) * 4):
                    o_sb = outp.tile([P, D], F32, tag="osb")
                    for ob in range(2):
                        po_t = big_psum()
                        po = po_t[:, 0, :]
                        for ct in range(2):
                            nc.tensor.matmul(
                                po[:],
                                yT[:, ct, qt * P:(qt + 1) * P],
                                wo_sb[:, ct, ob * 512:(ob + 1) * 512],
                                start=(ct == 0), stop=(ct == 1),
                            )
                        nc.vector.tensor_copy(o_sb[:, ob * 512:(ob + 1) * 512],
                                              po[:])
                    nc.scalar.dma_start(out[qt * P:(qt + 1) * P, :], o_sb[:])


_NC_CACHE = {}


def _get_program():
    key = os.environ.get("KVARIANT", "full")
    if key not in _NC_CACHE:
        _NC_CACHE[key] = _build_program()
    return _NC_CACHE[key]


def _in_maps(queries, keys, values, Wq, Wk, Wv, Wo):
    maps = []
    for c in range(N_CORES):
        b, g = divmod(c, 4)
        cs = slice(g * C, (g + 1) * C)
        maps.append({
            "xq": np.ascontiguousarray(queries[b].T, dtype=np.float32),
            "xk": np.ascontiguousarray(keys[b].T, dtype=np.float32),
            "xv": np.ascontiguousarray(values[b].T, dtype=np.float32),
            "wq": np.ascontiguousarray(Wq[:, cs], dtype=np.float32),
            "wk": np.ascontiguousarray(Wk[:, cs], dtype=np.float32),
            "wv": np.ascontiguousarray(Wv[:, cs], dtype=np.float32),
            "wo": np.ascontiguousarray(Wo[cs, :], dtype=np.float32),
        })
    return maps


def kernel(queries, keys, values, Wq, Wk, Wv, Wo):
    queries = np.asarray(queries)
    keys = np.asarray(keys)
    values = np.asarray(values)
    nc = _get_program()
    maps = _in_maps(queries, keys, values, np.asarray(Wq), np.asarray(Wk),
                    np.asarray(Wv), np.asarray(Wo))
    res = run_bass_kernel_spmd(nc, maps, list(range(N_CORES)))
    B = queries.shape[0]
    outs = []
    for b in range(B):
        acc = res.results[4 * b]["out"].astype(np.float32).copy()
        for g in range(1, 4):
            acc += res.results[4 * b + g]["out"]
        outs.append(acc)
    return np.stack(outs).astype(np.float32)


# revision 23
# speedup vs baseline: 1.0631x; 1.0631x over previous
"""Multi-head attention (B=2, S=2048, D=1024, 16 heads) on 8 Trainium2 NeuronCores.

Sharding: batch x head-group (data + tensor parallel). Core c handles batch
b = c // 4 and head group g = c % 4 (4 heads, feature columns g*256:(g+1)*256
of Wq/Wk/Wv column-split; rows g*256:(g+1)*256 of Wo row-split). Each core
computes a full [2048, 1024] partial of the output projection for its batch;
the host sums the 4 partials per batch (the Wo row-split reduction). The host
ships activations in feature-major ([D, S]) layout - the layout/sharding prep
done in kernel() alongside the per-core slicing - so the device contracts
over the partition dim directly.

Per-core dataflow (matmuls fp32r at full PE rate, fp32 PSUM accumulation):
  K^T, V (natural, ones-augmented), Q^T projections (K/V first so attention
  on early q-blocks overlaps the Q-projection tail)  ->  scores^T = K @ Q^T
  per head (row-packed pairs: two dh=64 heads share the 128-row PE array)
  ->  exp on ACT straight out of PSUM (1/sqrt(dh) fused into the activation
  scale)  ->  Y^T = [V | 1]^T @ expS^T (PSUM-accumulated; row 64 = softmax
  denominator)  ->  normalize by broadcast reciprocal  ->  out = Y @ Wo with
  Y^T stationary (natural output layout)  ->  DMA out.
"""

import os
import sys

for _p in ("/opt/trn_rl_repo", os.path.expanduser("~/.axon_site/_ro/trn_rl_repo")):
    if os.path.isdir(_p) and _p not in sys.path:
        sys.path.append(_p)

import numpy as np

import concourse.bass as bass
import concourse.tile as tile
from concourse import bacc, mybir
from concourse.bass_utils import run_bass_kernel_spmd
from concourse.masks import make_identity

F32 = mybir.dt.float32
FR = mybir.dt.float32r

S = 2048          # sequence length
D = 1024          # model dim
C = 256           # per-core feature columns (4 heads x 64)
H = 4             # local heads
DH = 64           # head dim
P = 128
NKB = D // P      # 8 contraction blocks for projections
NKT = S // P      # 16 key tiles
NQB = 4           # q blocks of 512
QB = S // NQB     # 512
NSB = 4           # s-blocks for the projection phase
SB = S // NSB     # 512
N_CORES = 8


def _build_program():
    nc = bacc.Bacc("TRN2", target_bir_lowering=False, debug=False,
                   num_devices=N_CORES)
    # activations arrive feature-major: [D, S]
    xq = nc.dram_tensor("xq", [D, S], F32, kind="ExternalInput").ap()
    xk = nc.dram_tensor("xk", [D, S], F32, kind="ExternalInput").ap()
    xv = nc.dram_tensor("xv", [D, S], F32, kind="ExternalInput").ap()
    wq = nc.dram_tensor("wq", [D, C], F32, kind="ExternalInput").ap()
    wk = nc.dram_tensor("wk", [D, C], F32, kind="ExternalInput").ap()
    wv = nc.dram_tensor("wv", [D, C], F32, kind="ExternalInput").ap()
    wo = nc.dram_tensor("wo", [C, D], F32, kind="ExternalInput").ap()
    out = nc.dram_tensor("out", [S, D], F32, kind="ExternalOutput").ap()

    with tile.TileContext(nc) as tc:
        _mha_tile(tc, xq, xk, xv, wq, wk, wv, wo, out)
    nc.compile()
    return nc


def _mha_tile(tc, xq, xk, xv, wq, wk, wv, wo, out):
    nc = tc.nc
    VAR = os.environ.get("KVARIANT", "full")
    reps = int(VAR[1:]) if VAR.startswith("x") else 1
    import contextlib
    ctx = contextlib.ExitStack()
    with ctx:
        consts = ctx.enter_context(tc.tile_pool(name="consts", bufs=1))
        xtp = ctx.enter_context(tc.tile_pool(name="xtp", bufs=3))
        acts = ctx.enter_context(tc.tile_pool(name="acts", bufs=1))
        expp = ctx.enter_context(tc.tile_pool(name="expp", bufs=4))
        outp = ctx.enter_context(tc.tile_pool(name="outp", bufs=3))
        small = ctx.enter_context(tc.tile_pool(name="small", bufs=2))
        # PSUM: tag "big" = 3 banks x 2 slots, tag "y" = 1 bank x 2 slots -> 8.
        psb = ctx.enter_context(tc.tile_pool(name="psb", bufs=2, space="PSUM"))
        psy = ctx.enter_context(tc.tile_pool(name="psy", bufs=2, space="PSUM"))

        def big_psum():
            return psb.tile([128, 3, 512], F32, tag="big", name="bigp")

        def y_psum():
            return psy.tile([128, 512], F32, tag="y", name="yp")

        # ---- weights to SBUF
        wq_sb = consts.tile([P, NKB, C], FR, tag="wq")
        nc.sync.dma_start(wq_sb[:],
                          wq.bitcast(FR).rearrange("(kb p) c -> p kb c", p=P))
        wk_sb = consts.tile([P, NKB, C], FR, tag="wk")
        nc.sync.dma_start(wk_sb[:],
                          wk.bitcast(FR).rearrange("(kb p) c -> p kb c", p=P))
        wv_sb = consts.tile([P, NKB, C], FR, tag="wv")
        nc.sync.dma_start(wv_sb[:],
                          wv.bitcast(FR).rearrange("(kb p) c -> p kb c", p=P))
        wo_sb = consts.tile([P, 2, D], FR, tag="wo")
        nc.sync.dma_start(wo_sb[:],
                          wo.bitcast(FR).rearrange("(ct p) n -> p ct n", p=P))

        # ---- persistent activations
        qT = acts.tile([P, 2, S], FR, tag="qT")     # Q^T: [c%128, ct, s]
        kT = acts.tile([P, 2, S], FR, tag="kT")     # K^T
        v_sb = acts.tile([P, NKT, H, DH + 1], FR, tag="v")  # V natural + ones
        yT = acts.tile([P, 2, S], FR, tag="yT")     # Y^T: [c%128, ct, q]
        ones = consts.tile([P, 1], F32, tag="ones")
        nc.vector.memset(ones[:], 1.0)
        nc.vector.tensor_copy(
            v_sb[:, :, :, DH:DH + 1],
            ones[:, :, None].to_broadcast((P, NKT * H, 1)).rearrange(
                "p (t h) o -> p t h o", h=H),
        )

        loop_n = int(VAR[1:]) if VAR.startswith("L") else 0
        loop_cm = tc.For_i(0, loop_n, 1) if loop_n else None
        if loop_cm is not None:
            ctx.enter_context(loop_cm)
        for _rep in range(reps):
            # ---- phase 1: projections (K, V, then Q; x^T streamed from HBM)
            for t, x_d in ((1, xk), (2, xv), (0, xq)):
                for sb in range(NSB):
                    ss = slice(sb * SB, (sb + 1) * SB)
                    xT = xtp.tile([P, NKB, SB], FR, tag="xT")
                    nc.sync.dma_start(
                        xT[:],
                        x_d.bitcast(FR).rearrange(
                            "(kb p) s -> p kb s", p=P)[:, :, ss],
                    )
                    if t < 2:  # Q^T / K^T : feature-major
                        dst = qT if t == 0 else kT
                        w_sb = wq_sb if t == 0 else wk_sb
                        for ct in range(2):
                            pq = big_psum()
                            for kb in range(NKB):
                                nc.tensor.matmul(
                                    pq[:, 0, :],
                                    w_sb[:, kb, ct * P:(ct + 1) * P],
                                    xT[:, kb, :],
                                    start=(kb == 0),
                                    stop=(kb == NKB - 1),
                                )
                            nc.any.tensor_copy(dst[:, ct, ss], pq[:, 0, :])
                    else:  # V natural, scattered into per-head 65-col slots
                        for st in range(4):
                            pv = big_psum()
                            for kb in range(NKB):
                                nc.tensor.matmul(
                                    pv[:, 0, 0:C],
                                    xT[:, kb, st * P:(st + 1) * P],
                                    wv_sb[:, kb, :],
                                    start=(kb == 0),
                                    stop=(kb == NKB - 1),
                                )
                            nc.any.tensor_copy(
                                v_sb[:, sb * 4 + st, :, 0:DH],
                                pv[:, 0, 0:C].rearrange(
                                    "p (h d) -> p h d", h=H),
                            )

            # ---- phase 2: attention + output projection
            KT_GROUPS = [(0, 3), (3, 6), (6, 9), (9, 12), (12, 15), (15, 16)]
            for qb in range(NQB):
                qs = slice(qb * QB, (qb + 1) * QB)
                for pair in range(2):  # heads (2*pair, 2*pair+1)
                    ya = y_psum()
                    yb = y_psum()
                    for g0, g1 in KT_GROUPS:
                        gn = g1 - g0
                        sa = big_psum()
                        sb_ = big_psum()
                        for j, kt in enumerate(range(g0, g1)):
                            ks = slice(kt * P, (kt + 1) * P)
                            nc.tensor.matmul(
                                sa[:, j, :], kT[0:DH, pair, ks],
                                qT[0:DH, pair, qs],
                                start=True, stop=True, tile_position=(0, 0),
                            )
                            nc.tensor.matmul(
                                sb_[:, j, :], kT[DH:P, pair, ks],
                                qT[DH:P, pair, qs],
                                start=True, stop=True, tile_position=(64, 0),
                            )
                        ea = expp.tile([P, 3, 512], FR, tag="exp")
                        eb = expp.tile([P, 3, 512], FR, tag="exp")
                        if VAR == "noexp":
                            nc.vector.tensor_copy(ea[:, 0:gn, :],
                                                  sa[:, 0:gn, :])
                            nc.vector.tensor_copy(eb[:, 0:gn, :],
                                                  sb_[:, 0:gn, :])
                        else:
                            nc.scalar.activation(
                                ea[:, 0:gn, :], sa[:, 0:gn, :],
                                mybir.ActivationFunctionType.Exp, scale=0.125)
                            nc.scalar.activation(
                                eb[:, 0:gn, :], sb_[:, 0:gn, :],
                                mybir.ActivationFunctionType.Exp, scale=0.125)
                        for j, kt in enumerate(range(g0, g1)):
                            nc.tensor.matmul(
                                ya[0:DH + 1, :],
                                v_sb[:, kt, 2 * pair, :],
                                ea[:, j, :],
                                start=(kt == 0), stop=(kt == NKT - 1),
                            )
                            nc.tensor.matmul(
                                yb[0:DH + 1, :],
                                v_sb[:, kt, 2 * pair + 1, :],
                                eb[:, j, :],
                                start=(kt == 0), stop=(kt == NKT - 1),
                            )
                    # normalize: rows 0..63 / broadcast(row 64)
                    for a, yp in ((0, ya), (1, yb)):
                        rc = small.tile([1, 512], F32, tag="rc")
                        nc.vector.reciprocal(rc[:], yp[DH:DH + 1, :])
                        rcb = small.tile([DH, 512], F32, tag="rcb")
                        nc.gpsimd.partition_broadcast(rcb[:], rc[:])
                        nc.vector.tensor_mul(
                            yT[a * DH:(a + 1) * DH, pair, qs],
                            yp[0:DH, :], rcb[:])

                # output projection for this q block (both pairs now done)
                for qt in range(qb * 4, (qb + 1) * 4):
                    o_sb = outp.tile([P, D], F32, tag="osb")
                    for ob in range(2):
                        po = y_psum()
                        for ct in range(2):
                            nc.tensor.matmul(
                                po[:],
                                yT[:, ct, qt * P:(qt + 1) * P],
                                wo_sb[:, ct, ob * 512:(ob + 1) * 512],
                                start=(ct == 0), stop=(ct == 1),
                            )
                        nc.vector.tensor_copy(o_sb[:, ob * 512:(ob + 1) * 512],
                                              po[:])
                    nc.sync.dma_start(out[qt * P:(qt + 1) * P, :], o_sb[:])


_NC_CACHE = {}


def _get_program():
    key = os.environ.get("KVARIANT", "full")
    if key not in _NC_CACHE:
        _NC_CACHE[key] = _build_program()
    return _NC_CACHE[key]


def _in_maps(queries, keys, values, Wq, Wk, Wv, Wo):
    maps = []
    for c in range(N_CORES):
        b, g = divmod(c, 4)
        cs = slice(g * C, (g + 1) * C)
        maps.append({
            "xq": np.ascontiguousarray(queries[b].T, dtype=np.float32),
            "xk": np.ascontiguousarray(keys[b].T, dtype=np.float32),
            "xv": np.ascontiguousarray(values[b].T, dtype=np.float32),
            "wq": np.ascontiguousarray(Wq[:, cs], dtype=np.float32),
            "wk": np.ascontiguousarray(Wk[:, cs], dtype=np.float32),
            "wv": np.ascontiguousarray(Wv[:, cs], dtype=np.float32),
            "wo": np.ascontiguousarray(Wo[cs, :], dtype=np.float32),
        })
    return maps


def kernel(queries, keys, values, Wq, Wk, Wv, Wo):
    queries = np.asarray(queries)
    keys = np.asarray(keys)
    values = np.asarray(values)
    nc = _get_program()
    maps = _in_maps(queries, keys, values, np.asarray(Wq), np.asarray(Wk),
                    np.asarray(Wv), np.asarray(Wo))
    res = run_bass_kernel_spmd(nc, maps, list(range(N_CORES)))
    B = queries.shape[0]
    outs = []
    for b in range(B):
        acc = res.results[4 * b]["out"].astype(np.float32).copy()
        for g in range(1, 4):
            acc += res.results[4 * b + g]["out"]
        outs.append(acc)
    return np.stack(outs).astype(np.float32)
